# revision 40
# baseline (speedup 1.0000x reference)
"""Trainium2 Bass kernel for a Transformer-XL style BertLayer (relative attention).

Sharding (8 NeuronCores, full inputs in / full output out):
  Dispatch 1: token-sharded transposed projections qT/kT/vT/pT in bf16.
  Host: reassemble; add pos_bias_u/v; build fp8 operands (zero-padded
    K=128 stationaries for BD/AC, DoubleRow-paired V / W1 / W2 / Wo);
    query-split for dispatch 2 (core c: batch c//4, queries
    [512*(c%4), +512)).
  Dispatch 2: attention with keys-on-partitions. The rel-shift is done by
    writing the dense BD position-score matrix to DRAM (rect, fp8) and
    reading it back through a sheared flat access pattern with an
    fp8->f32 casting SWDGE DMA, then PE-transposing into the scores^T
    PSUM accumulation on top of the content scores. Softmax denominators
    ride as a ones-column appended to V (fp8 DoubleRow matmul). Then
    Wo (fp8 DoubleRow) + residual/LN1 + FFN in fp8 DoubleRow (exact
    GELU) + residual/LN2.
"""

import os
import sys
import numpy as np
import ml_dtypes

sys.path.insert(0, "/opt/trn_rl_repo")

import concourse.bass as bass
import concourse.mybir as mybir
import concourse.tile as tile
from concourse import bacc
from concourse.bass_utils import run_bass_kernel_spmd
from concourse.masks import make_identity

BF = ml_dtypes.bfloat16
F8 = ml_dtypes.float8_e4m3
F32, BF16, F32R = mybir.dt.float32, mybir.dt.bfloat16, mybir.dt.float32r
FP8 = mybir.dt.float8e4
DR = mybir.MatmulPerfMode.DoubleRow
AFT = mybir.ActivationFunctionType
ALU = mybir.AluOpType
AXX = mybir.AxisListType.X

B, T, H, NH, DK = 2, 2048, 768, 12, 64
P = 128
FC = H // P            # 6 feature chunks
GC = 3072 // P         # 24 intermediate chunks
Q = 512                # queries per core
NCORE = 8
WWIN = 2560            # pT window width per core
BDW = 2176             # BD rect row width (2175 used + 1 pad)
LN_EPS = 1e-5
WSC = 16.0             # fp8 weight pre-scale (host side)
CSC = 32.0             # ctxT pre-scale

_cache = {}
PROFILE = {}


def _build_d1():
    nc = bacc.Bacc(None, target_bir_lowering=False)
    xT = nc.dram_tensor("xT", [P, FC, Q], FP8, kind="ExternalInput")
    posT = nc.dram_tensor("posT", [P, FC, Q], FP8, kind="ExternalInput")
    ws = {n: nc.dram_tensor(n, [P, FC, 3, 2, P], FP8, kind="ExternalInput")
          for n in ("Wq", "Wk", "Wv", "Wp")}
    bs = {n: nc.dram_tensor(n, [P, FC], F32, kind="ExternalInput")
          for n in ("bq", "bk", "bv")}
    outs = {n: nc.dram_tensor(n, [P, FC, Q], BF16, kind="ExternalOutput")
            for n in ("qT", "kT", "vT", "pT")}

    with tile.TileContext(nc) as tc:
        with tc.tile_pool(name="sb", bufs=2) as sb, \
             tc.tile_pool(name="wp", bufs=2) as wp, \
             tc.tile_pool(name="ps", bufs=3, space="PSUM") as psp:
            xT_sb = sb.tile([P, FC, Q], FP8, tag="x")
            nc.sync.dma_start(xT_sb[:], xT[:])
            posT_sb = sb.tile([P, FC, Q], FP8, tag="p")
            nc.scalar.dma_start(posT_sb[:], posT[:])
            bias_sb = {}
            for n in bs:
                t = sb.tile([P, FC], F32, tag=n)
                nc.scalar.dma_start(t[:], bs[n][:])
                bias_sb[n] = t

            ev = 0
            for wn, bn, on, src in (("Wq", "bq", "qT", xT_sb),
                                    ("Wk", "bk", "kT", xT_sb),
                                    ("Wv", "bv", "vT", xT_sb),
                                    ("Wp", None, "pT", posT_sb)):
                w_sb = wp.tile([P, FC, 3, 2, P], FP8, tag="w")
                (nc.sync if ev % 2 == 0 else nc.scalar).dma_start(
                    w_sb[:], ws[wn][:])
                ev += 1
                o_sb = sb.tile([P, FC, Q], BF16, tag="o")
                for dc in range(FC):
                    ps = psp.tile([P, Q], F32, tag="ps")
                    for fcp in range(3):
                        nc.tensor.matmul(ps[:], w_sb[:, dc, fcp],
                                         src[:, 2 * fcp:2 * fcp + 2, :],
                                         perf_mode=DR,
                                         start=(fcp == 0), stop=(fcp == 2))
                    if bn is None:
                        nc.scalar.activation(o_sb[:, dc], ps[:], AFT.Copy,
                                             scale=1.0 / WSC)
                    else:
                        nc.scalar.activation(o_sb[:, dc], ps[:], AFT.Identity,
                                             scale=1.0 / WSC,
                                             bias=bias_sb[bn][:, dc:dc + 1])
                nc.sync.dma_start(outs[on][:], o_sb[:])
    nc.compile()
    return nc


def _build_d2(use_mask: bool, affine: bool, debug: bool = False):
    nc = bacc.Bacc(None, target_bir_lowering=False)
    qvTz = nc.dram_tensor("qvTz", [P, NH, 4, P], FP8, kind="ExternalInput")
    pTw = nc.dram_tensor("pTw", [P, FC, WWIN], FP8, kind="ExternalInput")
    quT = nc.dram_tensor("quT", [P, FC, Q], FP8, kind="ExternalInput")
    kTz = nc.dram_tensor("kTz", [P, NH, 16, P], FP8, kind="ExternalInput")
    vb2 = nc.dram_tensor("vb2", [P, NH, 8, 2, 96], FP8, kind="ExternalInput")
    Wo = nc.dram_tensor("Wo", [P, FC, H], FP8, kind="ExternalInput")
    W1 = nc.dram_tensor("W1", [P, FC, 3072], BF16, kind="ExternalInput")
    W2 = nc.dram_tensor("W2", [P, GC, H], BF16, kind="ExternalInput")
    b1c = nc.dram_tensor("b1c", [P, GC], F32, kind="ExternalInput")
    xq = nc.dram_tensor("xq", [P, 4, H], F32, kind="ExternalInput")
    if affine:
        # rows: 0=bo 1=b2 2=ln1_g 3=ln1_b 4=ln2_g 5=ln2_b (replicated over partitions)
        vecs = nc.dram_tensor("vecs", [P, 6, H], F32, kind="ExternalInput")
    if use_mask:
        maskb = nc.dram_tensor("maskb", [P, 16, Q], FP8, kind="ExternalInput")
        keepb = nc.dram_tensor("keepb", [P, 16, Q], FP8, kind="ExternalInput")
    out = nc.dram_tensor("out", [P, 4, H], F32, kind="ExternalOutput")
    if debug:
        dbg_rect = nc.dram_tensor("dbg_rect", [P, BDW], FP8, kind="ExternalOutput")
        dbg_bdsh = nc.dram_tensor("dbg_bdsh", [P, 4, 512], F32, kind="ExternalOutput")
        dbg_st = nc.dram_tensor("dbg_st", [P, 1024], F32, kind="ExternalOutput")
        dbg_e2 = nc.dram_tensor("dbg_e2", [P, 2, Q], FP8, kind="ExternalOutput")
        dbg_ctx = nc.dram_tensor("dbg_ctx", [DK + 1, Q], F32, kind="ExternalOutput")
        dbg_ctxT = nc.dram_tensor("dbg_ctxT", [P, FC, Q], FP8, kind="ExternalOutput")
        dbg_x1 = nc.dram_tensor("dbg_x1", [P, 4, H], F32, kind="ExternalOutput")
        dbg_h1 = nc.dram_tensor("dbg_h1", [P, GC, Q], BF16, kind="ExternalOutput")

    wb = 1 if (use_mask or affine or debug) else 2
    with tile.TileContext(nc) as tc:
        with tc.tile_pool(name="res", bufs=1) as res, \
             tc.tile_pool(name="stream", bufs=3) as stream, \
             tc.tile_pool(name="work", bufs=2) as work, \
             tc.tile_pool(name="dram", bufs=3, space="DRAM") as dpool:

            # ---------------- resident loads (attention-critical first) ----
            qvTz_sb = res.tile([P, NH, 4, P], FP8, tag="qvTz")
            nc.sync.dma_start(qvTz_sb[:, 0], qvTz[:, 0])
            pT_sb = res.tile([P, FC, WWIN], FP8, tag="pTw")
            nc.sync.dma_start(pT_sb[:, 0], pTw[:, 0])
            quT_sb0 = None  # placeholder to keep structure clear
            kTz_h0 = stream.tile([P, 16, P], FP8, tag="kTzh", bufs=2)
            nc.sync.dma_start(kTz_h0[:], kTz[:, 0])
            vb_h0 = stream.tile([P, 8, 2, 96], FP8, tag="vbh", bufs=2)
            nc.scalar.dma_start(vb_h0[:], vb2[:, 0])
            nc.sync.dma_start(qvTz_sb[:, 1:], qvTz[:, 1:])
            nc.scalar.dma_start(pT_sb[:, 1:], pTw[:, 1:])
            quT_sb = res.tile([P, FC, Q], FP8, tag="quT")
            nc.sync.dma_start(quT_sb[:], quT[:])
            if use_mask:
                mb_sb = res.tile([P, 16, Q], FP8, tag="maskb")
                nc.sync.dma_start(mb_sb[:], maskb[:])
                kb_sb = res.tile([P, 16, Q], FP8, tag="keepb")
                nc.sync.dma_start(kb_sb[:], keepb[:])
            Wo_sb = res.tile([P, FC, H], FP8, tag="Wo")
            nc.sync.dma_start(Wo_sb[:], Wo[:])
            b1_sb = res.tile([P, GC], F32, tag="b1c")
            nc.sync.dma_start(b1_sb[:], b1c[:])
            if affine:
                vecs_sb = res.tile([P, 6, H], F32, tag="vecs")
                nc.sync.dma_start(vecs_sb[:], vecs[:])

            ident_raw = res.tile([P, P], F32, tag="idraw")
            make_identity(nc, ident_raw[:])
            ident = res.tile([P, P], F32R, tag="ident")
            nc.vector.tensor_copy(out=ident[:], in_=ident_raw[:])
            ident_bf = res.tile([P, P], BF16, tag="identbf")
            nc.vector.tensor_copy(out=ident_bf[:], in_=ident_raw[:])
            eps_sb = res.tile([P, 1], F32, tag="eps")
            nc.any.memset(eps_sb[:], LN_EPS)
            ones_f = res.tile([1, DK], F32, tag="onesf")
            nc.any.memset(ones_f[:], 1.0)
            ones_r = res.tile([1, DK], F32R, tag="onesr")
            nc.vector.tensor_copy(out=ones_r[:], in_=ones_f[:])

            ctxT = res.tile([P, FC, Q], FP8, tag="ctxT")

            # ---------------- attention ----------------
            with tc.tile_pool(name="ps_bd", bufs=2, space="PSUM") as ps_bd, \
                 tc.tile_pool(name="ps_st", bufs=2, space="PSUM") as ps_st, \
                 tc.tile_pool(name="ps_ctx", bufs=1, space="PSUM") as ps_ctx, \
                 tc.tile_pool(name="ps_bc", bufs=1, space="PSUM") as ps_bc:
                for h in range(NH):
                    hp, hc = DK * (h % 2), h // 2
                    if h == 0:
                        kTz_h, vb_h = kTz_h0, vb_h0
                    else:
                        kTz_h = stream.tile([P, 16, P], FP8, tag="kTzh",
                                            bufs=2)
                        nc.sync.dma_start(kTz_h[:], kTz[:, h])
                        vb_h = stream.tile([P, 8, 2, 96], FP8, tag="vbh",
                                           bufs=2)
                        nc.scalar.dma_start(vb_h[:], vb2[:, h])
                    # --- stage A: dense BD rect (q part, r free) -> DRAM fp8
                    bdd = dpool.tile([4, P, BDW], FP8, tag="bdd")
                    ev = 0
                    for qt in range(4):
                        loc = 384 - 128 * qt
                        bd_sb = work.tile([P, BDW], FP8, tag="bd_sb")
                        for rc in range(5):
                            w = 512 if rc < 4 else 128
                            ps = ps_bd.tile([P, 512], F32, tag="bd")
                            nc.tensor.matmul(
                                ps[:, :w],
                                qvTz_sb[:, h, qt],
                                pT_sb[:, hc,
                                      loc + rc * 512: loc + rc * 512 + w],
                                start=True, stop=True)
                            if ev % 2 == 0:
                                nc.vector.tensor_copy(
                                    out=bd_sb[:, rc * 512: rc * 512 + w],
                                    in_=ps[:, :w])
                            else:
                                nc.scalar.activation(
                                    bd_sb[:, rc * 512: rc * 512 + w],
                                    ps[:, :w], AFT.Copy)
                            ev += 1
                        (nc.sync if qt % 2 == 0 else nc.scalar).dma_start(
                            bdd[qt], bd_sb[:])
                        if debug and h == 0 and qt == 0:
                            nc.sync.dma_start(dbg_rect[:], bd_sb[:])

                    # --- stage B: sheared fp8->f32 read + scores^T + exp + V
                    ctx = ps_ctx.tile([96, Q], F32, tag="ctx")
                    for jq in range(4):
                        bdsh = stream.tile(
                            [P, 4, 512], F32R, tag="bdsh",
                            bufs=2)
                        src = bass.AP(bdd.tensor, bdd.offset + 127 + 512 * jq,
                                      [[BDW - 1, P], [P * BDW, 4], [1, 512]])
                        nc.gpsimd.dma_start(bdsh[:], src)
                        if debug and h == 0 and jq == 0:
                            bdsh_c = work.tile([P, 4, 512], F32, tag="bdshc")
                            nc.vector.tensor_copy(out=bdsh_c[:],
                                                  in_=bdsh[:].bitcast(F32))
                            nc.sync.dma_start(dbg_bdsh[:], bdsh_c[:])
                        for jph in range(2):
                            jp = 2 * jq + jph
                            st = ps_st.tile([P, 1024], F32, tag="st")
                            for half in range(2):
                                jt = 2 * jp + half
                                co = 512 * half
                                for qt in range(4):
                                    nc.tensor.matmul(
                                        st[:, co + qt * P: co + (qt + 1) * P
                                           ].bitcast(F32R),
                                        bdsh[:, qt,
                                             jph * 256 + half * P:
                                             jph * 256 + (half + 1) * P],
                                        ident[:], is_transpose=True,
                                        start=(qt == 0), stop=False)
                                nc.tensor.matmul(
                                    st[:, co:co + 512],
                                    kTz_h[:, jt],
                                    quT_sb[:, hc, :],
                                    start=False, stop=True)
                                if use_mask:
                                    nc.vector.tensor_tensor(
                                        st[:, co:co + 512], st[:, co:co + 512],
                                        mb_sb[:, jt, :], ALU.add)
                            if debug and h == 0 and jp == 0:
                                st_c = work.tile([P, 1024], F32, tag="stc")
                                nc.vector.tensor_copy(out=st_c[:], in_=st[:])
                                nc.sync.dma_start(dbg_st[:], st_c[:])
                            e2 = work.tile([P, 2, Q], FP8, tag="e2")
                            nc.scalar.activation(e2[:, 0], st[:, :512],
                                                 AFT.Exp, scale=0.125)
                            nc.scalar.activation(e2[:, 1], st[:, 512:],
                                                 AFT.Exp, scale=0.125)
                            if use_mask:
                                for half in range(2):
                                    jt = 2 * jp + half
                                    nc.vector.tensor_tensor(
                                        e2[:, half], e2[:, half],
                                        kb_sb[:, jt, :], ALU.mult)
                            if debug and h == 0 and jp == 0:
                                nc.sync.dma_start(dbg_e2[:], e2[:])
                            nc.tensor.matmul(ctx[:], vb_h[:, jp], e2[:],
                                             perf_mode=DR,
                                             start=(jp == 0), stop=(jp == 7))

                    # --- stage C: normalize by softmax denominator
                    ctx_sb = work.tile([DK + 1, Q], F32, tag="ctx_sb")
                    nc.scalar.activation(ctx_sb[:], ctx[:DK + 1], AFT.Copy)
                    den_s = work.tile([1, Q], F32, tag="den_s", bufs=wb)
                    nc.scalar.mul(den_s[:], ctx_sb[DK:DK + 1, :], 1.0 / CSC)
                    den_i = work.tile([1, Q], F32, tag="den_i", bufs=wb)
                    nc.vector.reciprocal(den_i[:], den_s[:])
                    den_r = work.tile([1, Q], F32R, tag="den_r", bufs=wb)
                    nc.vector.tensor_copy(out=den_r[:], in_=den_i[:])
                    bc = ps_bc.tile([DK, Q], F32, tag="bc")
                    nc.tensor.matmul(bc[:], ones_r[:], den_r[:],
                                     start=True, stop=True)
                    nc.vector.tensor_tensor(ctxT[hp:hp + DK, hc, :],
                                            ctx_sb[:DK, :], bc[:], ALU.mult)
                    if debug and h == 0:
                        ctx_c = work.tile([DK + 1, Q], F32, tag="ctxc")
                        nc.scalar.activation(ctx_c[:], ctx[:DK + 1], AFT.Copy)
                        nc.sync.dma_start(dbg_ctx[:], ctx_c[:])

            # ---------------- Wo + LN1 + FFN + LN2 ----------------
            if debug:
                nc.sync.dma_start(dbg_ctxT[:], ctxT[:])
            x1f = res.tile([P, 4, H], F32, tag="x1f")
            x1T = res.tile([P, FC, Q], BF16, tag="x1T")

            def layer_norm(dst, src, g_row, b_row, bf_copy=None):
                # dst = LN(src) [* g + b]; src is an f32 (P, H) SBUF AP, may alias
                s1 = work.tile([P, 1], F32, tag="s1")
                nc.vector.tensor_reduce(s1[:], src, axis=AXX, op=ALU.add)
                nm = work.tile([P, 1], F32, tag="nm")
                nc.scalar.mul(nm[:], s1[:], -1.0 / H)
                nc.vector.tensor_scalar(src, src, nm[:], None, op0=ALU.add)
                sq = work.tile([P, H], F32, tag="sq", bufs=wb)
                s2 = work.tile([P, 1], F32, tag="s2")
                nc.scalar.activation(sq[:], src, AFT.Square, accum_out=s2[:])
                lnv = work.tile([P, 1], F32, tag="lnv")
                nc.scalar.activation(lnv[:], s2[:], AFT.Ln, scale=1.0 / H,
                                     bias=eps_sb[:, 0:1])
                rstd = work.tile([P, 1], F32, tag="rstd")
                nc.scalar.activation(rstd[:], lnv[:], AFT.Exp, scale=-0.5)
                if affine:
                    nc.vector.tensor_scalar(src, src, rstd[:], None, op0=ALU.mult)
                    nc.vector.tensor_tensor(sq[:], src, g_row, ALU.mult)
                    nc.vector.tensor_tensor(dst, sq[:], b_row, ALU.add)
                    if bf_copy is not None:
                        nc.vector.tensor_copy(out=bf_copy, in_=dst)
                elif bf_copy is not None:
                    # bf16 result first (unblocks the x1T transposes), then
                    # the f32 copy for the LN2 residual off the critical path
                    nc.vector.tensor_scalar(bf_copy, src, rstd[:], None,
                                            op0=ALU.mult)
                    nc.vector.tensor_copy(out=dst, in_=bf_copy)
                else:
                    nc.vector.tensor_scalar(dst, src, rstd[:], None, op0=ALU.mult)

            with tc.tile_pool(name="ps_ao", bufs=2, space="PSUM") as ps_ao, \
                 tc.tile_pool(name="ps_tr", bufs=2, space="PSUM") as ps_tr:
                for qt in range(4):
                    ao = ps_ao.tile([P, H], F32, tag="ao")
                    for hcp in range(FC // 2):
                        for c0, c1 in ((0, 512), (512, 768)):
                            nc.tensor.matmul(
                                ao[:, c0:c1],
                                ctxT[:, 2 * hcp:2 * hcp + 2,
                                     qt * P:(qt + 1) * P],
                                Wo_sb[:, 2 * hcp:2 * hcp + 2, c0:c1],
                                perf_mode=DR,
                                start=(hcp == 0), stop=(hcp == FC // 2 - 1))
                    ao_s = work.tile([P, H], F32, tag="ao_s", bufs=wb)
                    nc.scalar.activation(ao_s[:], ao[:], AFT.Copy,
                                         scale=1.0 / (WSC * CSC))
                    xqt = stream.tile([P, H], F32, tag="xqt", bufs=2)
                    nc.sync.dma_start(xqt[:], xq[:, qt])
                    resid = work.tile([P, H], F32, tag="resid")
                    nc.vector.tensor_tensor(resid[:], ao_s[:], xqt[:],
                                            ALU.add)
                    if affine:
                        nc.vector.tensor_tensor(resid[:], resid[:],
                                                vecs_sb[:, 0], ALU.add)
                    x1b = work.tile([P, H], BF16, tag="x1b")
                    layer_norm(x1f[:, qt], resid[:],
                               vecs_sb[:, 2] if affine else None,
                               vecs_sb[:, 3] if affine else None,
                               bf_copy=x1b[:])
                    for fcc in range(FC):
                        tp = ps_tr.tile([P, P], BF16, tag="tr")
                        nc.tensor.matmul(tp[:], x1b[:, fcc * P:(fcc + 1) * P],
                                         ident_bf[:], is_transpose=True,
                                         start=True, stop=True)
                        nc.vector.tensor_copy(
                            out=x1T[:, fcc, qt * P:(qt + 1) * P], in_=tp[:])

            if debug:
                nc.sync.dma_start(dbg_x1[:], x1f[:])
            # FFN1 + FFN2 interleaved: FFN2 accumulates into SBUF per
            # 6-gc chunk so its PSUM pool coexists with FFN1's.
            h1_tiles = [res.tile([P, Q], BF16, tag=f"h1_{gc}",
                                 name=f"h1_{gc}")
                        for gc in range(GC)]
            o_acc = [res.tile([P, H], F32, tag=f"oa{qt}",
                              name=f"oa{qt}") for qt in range(4)]
            CH = 6
            with tc.tile_pool(name="ps_h1", bufs=2, space="PSUM") as ps_h1, \
                 tc.tile_pool(name="ps_o2", bufs=2, space="PSUM") as ps_o2:
                for cc in range(GC // CH):
                    w2c = stream.tile([P, CH, H], BF16, tag="w2c", bufs=wb)
                    nc.sync.dma_start(w2c[:], W2[:, cc * CH:(cc + 1) * CH, :])
                    for gi in range(CH):
                        gc = cc * CH + gi
                        w1s = stream.tile([P, FC, P], BF16, tag="w1s",
                                          bufs=2)
                        nc.scalar.dma_start(
                            w1s[:], W1[:, :, gc * P:(gc + 1) * P])
                        hp1 = ps_h1.tile([P, Q], F32, tag="h1")
                        for fc in range(FC):
                            nc.tensor.matmul(hp1[:], w1s[:, fc],
                                             x1T[:, fc, :],
                                             start=(fc == 0),
                                             stop=(fc == FC - 1))
                        nc.scalar.activation(h1_tiles[gc][:], hp1[:],
                                             AFT.Gelu,
                                             bias=b1_sb[:, gc:gc + 1])
                    for qt in range(4):
                        po = ps_o2.tile([P, H], F32, tag="po")
                        for gi in range(CH):
                            gc = cc * CH + gi
                            nc.tensor.matmul(
                                po[:, :512],
                                h1_tiles[gc][:, qt * P:(qt + 1) * P],
                                w2c[:, gi, :512],
                                start=(gi == 0), stop=(gi == CH - 1))
                            nc.tensor.matmul(
                                po[:, 512:],
                                h1_tiles[gc][:, qt * P:(qt + 1) * P],
                                w2c[:, gi, 512:],
                                start=(gi == 0), stop=(gi == CH - 1))
                        if cc == 0:
                            nc.vector.tensor_copy(out=o_acc[qt][:],
                                                  in_=po[:])
                        else:
                            nc.vector.tensor_tensor(
                                o_acc[qt][:], o_acc[qt][:], po[:],
                                ALU.add)

            if debug:
                for gc in range(GC):
                    nc.sync.dma_start(dbg_h1[:, gc], h1_tiles[gc][:])
            # LN2
            if True:
                for qt in range(4):
                    r2 = work.tile([P, H], F32, tag="resid2", bufs=wb)
                    nc.vector.tensor_tensor(r2[:], o_acc[qt][:], x1f[:, qt],
                                            ALU.add)
                    if affine:
                        nc.vector.tensor_tensor(r2[:], r2[:],
                                                vecs_sb[:, 1], ALU.add)
                    o_sb = work.tile([P, H], F32, tag="osb")
                    layer_norm(o_sb[:], r2[:],
                               vecs_sb[:, 4] if affine else None,
                               vecs_sb[:, 5] if affine else None)
                    nc.sync.dma_start(out[:, qt], o_sb[:])
    nc.compile()
    return nc


# --------------------------------------------------------------------------
def _chunk_pf(w):
    """(768, X) -> (128, 6, X) with row d' = 128*chunk + partition."""
    return np.ascontiguousarray(w.reshape(FC, P, -1).transpose(1, 0, 2))


def kernel(hidden_states, attention_mask, pos_emb,
           Wq, bq, Wk, bk, Wv, bv, Wp, pos_bias_u, pos_bias_v, Wo, bo,
           ln1_g, ln1_b, W1, b1, W2, b2, ln2_g, ln2_b):
    f32 = lambda x: np.asarray(x, dtype=np.float32)
    hidden_states = f32(hidden_states)
    pos_emb = f32(pos_emb)
    mask = np.asarray(attention_mask)
    use_mask = bool(mask.any())
    affine = not (np.all(f32(ln1_g) == 1) and np.all(f32(ln1_b) == 0)
                  and np.all(f32(ln2_g) == 1) and np.all(f32(ln2_b) == 0)
                  and np.all(f32(bo) == 0) and np.all(f32(b2) == 0))

    debug = bool(os.environ.get("BERT_KERNEL_DEBUG"))
    if "d1" not in _cache:
        _cache["d1"] = _build_d1()
    key = ("d2", use_mask, affine, debug)
    if key not in _cache:
        _cache[key] = _build_d2(use_mask, affine, debug)
    d1, d2 = _cache["d1"], _cache[key]

    hf = hidden_states.reshape(B * T, H)
    xT_full = _chunk_pf(np.ascontiguousarray(hf.T)).astype(F8)
    posT_pad = np.zeros((H, 4096), np.float32)
    posT_pad[:, :2 * T - 1] = pos_emb[0].T
    posT_full = _chunk_pf(posT_pad).astype(F8)

    def _w_d1(w):
        # [p, dc, fcp, t, m] = WSC * w[256*fcp + 128*t + p, 128*dc + m]
        return np.ascontiguousarray(
            (f32(w) * WSC).reshape(3, 2, P, FC, P).transpose(2, 3, 0, 1, 4)
        ).astype(F8)

    wq_c, wk_c, wv_c, wp_c = map(_w_d1, (Wq, Wk, Wv, Wp))
    bq_c = f32(bq).reshape(FC, P).T.copy()
    bk_c = f32(bk).reshape(FC, P).T.copy()
    bv_c = f32(bv).reshape(FC, P).T.copy()

    in1 = []
    for c in range(NCORE):
        sl = slice(512 * c, 512 * c + 512)
        in1.append({
            "xT": np.ascontiguousarray(xT_full[:, :, sl]),
            "posT": np.ascontiguousarray(posT_full[:, :, sl]),
            "Wq": wq_c, "Wk": wk_c, "Wv": wv_c, "Wp": wp_c,
            "bq": bq_c, "bk": bk_c, "bv": bv_c,
        })
    _trace = bool(os.environ.get("BERT_KERNEL_TRACE"))
    _res1 = run_bass_kernel_spmd(d1, in1, core_ids=list(range(NCORE)),
                                 trace=_trace)
    PROFILE["d1_ns"] = _res1.exec_time_ns
    PROFILE["d1_res"] = _res1
    r1 = _res1.results

    qT_full = np.concatenate([r["qT"] for r in r1], axis=2).astype(np.float32)
    kT_full = np.concatenate([r["kT"] for r in r1], axis=2).astype(np.float32)
    vT_full = np.concatenate([r["vT"] for r in r1], axis=2).astype(np.float32)
    pT_full = np.concatenate([r["pT"] for r in r1], axis=2).astype(np.float32)
    pT_full[:, :, 2 * T - 1:] = 0

    pbu_c = f32(pos_bias_u).reshape(NH * DK).reshape(FC, P).T.copy()
    pbv_c = f32(pos_bias_v).reshape(NH * DK).reshape(FC, P).T.copy()
    quT_full = (qT_full + pbu_c[:, :, None]).astype(F8)
    qvT_full = qT_full + pbv_c[:, :, None]                    # f32
    pT_f8 = pT_full.astype(F8)

    wo_c = (_chunk_pf(f32(Wo)) * WSC).astype(F8)
    w1_c = _chunk_pf(f32(W1)).astype(BF)
    w2_c = np.ascontiguousarray(
        f32(W2).reshape(GC, P, H).transpose(1, 0, 2)).astype(BF)
    b1_c = f32(b1).reshape(GC, P).T.copy()
    if affine:
        vecs = np.stack([np.broadcast_to(f32(x), (P, H)) for x in
                         (bo, b2, ln1_g, ln1_b, ln2_g, ln2_b)], axis=1).copy()

    in2 = []
    for c in range(NCORE):
        b_ = c // 4
        q0 = 512 * (c % 4)
        w0 = 1536 - q0
        tsl = slice(T * b_, T * b_ + T)

        # zero-padded K=128 stationaries (fp8), partition dim first
        qvTz = np.zeros((P, NH, 4, P), F8)
        kTz = np.zeros((P, NH, 16, P), F8)
        for h in range(NH):
            hp, hcc = DK * (h % 2), h // 2
            qvTz[hp:hp + DK, h] = qvT_full[
                hp:hp + DK, hcc, 512 * c:512 * c + 512
            ].reshape(DK, 4, P).astype(F8)
            kTz[hp:hp + DK, h] = kT_full[
                hp:hp + DK, hcc, tsl].reshape(DK, 16, P).astype(F8)

        vv = vT_full[:, :, tsl]                                   # (128,6,2048)
        vmat = np.ascontiguousarray(
            vv.transpose(1, 0, 2).reshape(H, T))                  # (768,2048)=v.T
        arr = vmat.reshape(NH, DK, 16, P).transpose(0, 3, 2, 1)   # (12,128,16,64)
        # vb2[p, h, jp, t, d] = arr[h, p, 2*jp + t, d]; ones column at d=64
        vb_c = np.zeros((P, NH, 8, 2, 96), F8)
        vb_c[:, :, :, :, :DK] = arr.reshape(NH, P, 8, 2, DK).transpose(
            1, 0, 2, 3, 4).astype(F8)
        vb_c[:, :, :, :, DK] = 1.0
        entry = {
            "qvTz": qvTz,
            "pTw": np.ascontiguousarray(pT_f8[:, :, w0:w0 + WWIN]),
            "quT": np.ascontiguousarray(quT_full[:, :, 512 * c:512 * c + 512]),
            "kTz": kTz,
            "vb2": vb_c,
            "xq": np.ascontiguousarray(
                hf[T * b_ + q0: T * b_ + q0 + 512].reshape(4, P, H)
                .transpose(1, 0, 2)),
            "Wo": wo_c, "W1": w1_c, "W2": w2_c, "b1c": b1_c,
        }
        if affine:
            entry["vecs"] = vecs
        if use_mask:
            m = f32(mask[b_])
            mT = m.T[:, q0:q0 + 512]                              # (2048,512) j,q
            entry["maskb"] = (mT.reshape(16, P, 512).transpose(1, 0, 2)
                              * np.float32(-240.0)).astype(F8)
            entry["keepb"] = (1.0 - mT.reshape(16, P, 512)
                              .transpose(1, 0, 2)).astype(F8)
        in2.append(entry)

    PROFILE["in2"] = in2
    _res2 = run_bass_kernel_spmd(d2, in2, core_ids=list(range(NCORE)),
                                 trace=_trace)
    PROFILE["d2_ns"] = _res2.exec_time_ns
    PROFILE["d2_res"] = _res2
    r2 = _res2.results

    outp = np.zeros((B, T, H), np.float32)
    for c in range(NCORE):
        b_ = c // 4
        q0 = 512 * (c % 4)
        outp[b_, q0:q0 + 512] = r2[c]["out"].transpose(1, 0, 2).reshape(512, H)
    return outp


# revision 41
# speedup vs baseline: 1.1583x; 1.1583x over previous
"""Trainium2 Bass kernel for a Transformer-XL style BertLayer (relative attention).

Sharding (8 NeuronCores, full inputs in / full output out):
  Dispatch 1: token-sharded transposed projections qT/kT/vT/pT in bf16.
  Host: reassemble; add pos_bias_u/v; build fp8 operands (zero-padded
    K=128 stationaries for BD/AC, DoubleRow-paired V / W1 / W2 / Wo);
    query-split for dispatch 2 (core c: batch c//4, queries
    [512*(c%4), +512)).
  Dispatch 2: attention with keys-on-partitions. The rel-shift is done by
    writing the dense BD position-score matrix to DRAM (rect, fp8) and
    reading it back through a sheared flat access pattern with an
    fp8->f32 casting SWDGE DMA, then PE-transposing into the scores^T
    PSUM accumulation on top of the content scores. Softmax denominators
    ride as a ones-column appended to V (fp8 DoubleRow matmul). Then
    Wo (fp8 DoubleRow) + residual/LN1 + FFN in fp8 DoubleRow (exact
    GELU) + residual/LN2.
"""

import os
import sys
import numpy as np
import ml_dtypes

sys.path.insert(0, "/opt/trn_rl_repo")

import concourse.bass as bass
import concourse.mybir as mybir
import concourse.tile as tile
from concourse import bacc
from concourse.bass_utils import run_bass_kernel_spmd
from concourse.masks import make_identity

BF = ml_dtypes.bfloat16
F8 = ml_dtypes.float8_e4m3
F32, BF16, F32R = mybir.dt.float32, mybir.dt.bfloat16, mybir.dt.float32r
FP8 = mybir.dt.float8e4
DR = mybir.MatmulPerfMode.DoubleRow
AFT = mybir.ActivationFunctionType
ALU = mybir.AluOpType
AXX = mybir.AxisListType.X

B, T, H, NH, DK = 2, 2048, 768, 12, 64
P = 128
FC = H // P            # 6 feature chunks
GC = 3072 // P         # 24 intermediate chunks
Q = 512                # queries per core
NCORE = 8
WWIN = 2560            # pT window width per core
BDW = 2176             # BD rect row width (2175 used + 1 pad)
LN_EPS = 1e-5
WSC = 16.0             # fp8 weight pre-scale (host side)
CSC = 32.0             # ctxT pre-scale

_cache = {}
PROFILE = {}


def _build_d1():
    nc = bacc.Bacc(None, target_bir_lowering=False)
    xT = nc.dram_tensor("xT", [P, FC, Q], FP8, kind="ExternalInput")
    posT = nc.dram_tensor("posT", [P, FC, Q], FP8, kind="ExternalInput")
    ws = {n: nc.dram_tensor(n, [P, FC, 3, 2, P], FP8, kind="ExternalInput")
          for n in ("Wq", "Wk", "Wv", "Wp")}
    bs = {n: nc.dram_tensor(n, [P, FC], F32, kind="ExternalInput")
          for n in ("bq", "bk", "bv")}
    outs = {n: nc.dram_tensor(n, [P, FC, Q], BF16, kind="ExternalOutput")
            for n in ("qT", "kT", "vT", "pT")}

    with tile.TileContext(nc) as tc:
        with tc.tile_pool(name="sb", bufs=2) as sb, \
             tc.tile_pool(name="wp", bufs=2) as wp, \
             tc.tile_pool(name="ps", bufs=3, space="PSUM") as psp:
            xT_sb = sb.tile([P, FC, Q], FP8, tag="x")
            nc.sync.dma_start(xT_sb[:], xT[:])
            posT_sb = sb.tile([P, FC, Q], FP8, tag="p")
            nc.scalar.dma_start(posT_sb[:], posT[:])
            bias_sb = {}
            for n in bs:
                t = sb.tile([P, FC], F32, tag=n)
                nc.scalar.dma_start(t[:], bs[n][:])
                bias_sb[n] = t

            ev = 0
            for wn, bn, on, src in (("Wq", "bq", "qT", xT_sb),
                                    ("Wk", "bk", "kT", xT_sb),
                                    ("Wv", "bv", "vT", xT_sb),
                                    ("Wp", None, "pT", posT_sb)):
                w_sb = wp.tile([P, FC, 3, 2, P], FP8, tag="w")
                (nc.sync if ev % 2 == 0 else nc.scalar).dma_start(
                    w_sb[:], ws[wn][:])
                ev += 1
                o_sb = sb.tile([P, FC, Q], BF16, tag="o")
                for dc in range(FC):
                    ps = psp.tile([P, Q], F32, tag="ps")
                    for fcp in range(3):
                        nc.tensor.matmul(ps[:], w_sb[:, dc, fcp],
                                         src[:, 2 * fcp:2 * fcp + 2, :],
                                         perf_mode=DR,
                                         start=(fcp == 0), stop=(fcp == 2))
                    if bn is None:
                        nc.scalar.activation(o_sb[:, dc], ps[:], AFT.Copy,
                                             scale=1.0 / WSC)
                    else:
                        nc.scalar.activation(o_sb[:, dc], ps[:], AFT.Identity,
                                             scale=1.0 / WSC,
                                             bias=bias_sb[bn][:, dc:dc + 1])
                nc.sync.dma_start(outs[on][:], o_sb[:])
    nc.compile()
    return nc


def _build_d2(use_mask: bool, affine: bool, debug: bool = False):
    nc = bacc.Bacc(None, target_bir_lowering=False)
    qvTz = nc.dram_tensor("qvTz", [P, NH, 4, P], FP8, kind="ExternalInput")
    pTw = nc.dram_tensor("pTw", [P, FC, WWIN], FP8, kind="ExternalInput")
    quT = nc.dram_tensor("quT", [P, FC, Q], FP8, kind="ExternalInput")
    kTz = nc.dram_tensor("kTz", [P, NH, 16, P], FP8, kind="ExternalInput")
    vb2 = nc.dram_tensor("vb2", [P, NH, 8, 2, 96], FP8, kind="ExternalInput")
    Wo = nc.dram_tensor("Wo", [P, FC, H], FP8, kind="ExternalInput")
    W1 = nc.dram_tensor("W1", [P, FC, 3072], BF16, kind="ExternalInput")
    W2 = nc.dram_tensor("W2", [P, GC, H], BF16, kind="ExternalInput")
    b1c = nc.dram_tensor("b1c", [P, GC], F32, kind="ExternalInput")
    xq = nc.dram_tensor("xq", [P, 4, H], F32, kind="ExternalInput")
    if affine:
        # rows: 0=bo 1=b2 2=ln1_g 3=ln1_b 4=ln2_g 5=ln2_b (replicated over partitions)
        vecs = nc.dram_tensor("vecs", [P, 6, H], F32, kind="ExternalInput")
    if use_mask:
        maskb = nc.dram_tensor("maskb", [P, 16, Q], FP8, kind="ExternalInput")
        keepb = nc.dram_tensor("keepb", [P, 16, Q], FP8, kind="ExternalInput")
    out = nc.dram_tensor("out", [P, 4, H], F32, kind="ExternalOutput")
    if debug:
        dbg_rect = nc.dram_tensor("dbg_rect", [P, BDW], FP8, kind="ExternalOutput")
        dbg_bdsh = nc.dram_tensor("dbg_bdsh", [P, 4, 512], F32, kind="ExternalOutput")
        dbg_st = nc.dram_tensor("dbg_st", [P, 1024], F32, kind="ExternalOutput")
        dbg_e2 = nc.dram_tensor("dbg_e2", [P, 2, Q], FP8, kind="ExternalOutput")
        dbg_ctx = nc.dram_tensor("dbg_ctx", [DK + 1, Q], F32, kind="ExternalOutput")
        dbg_ctxT = nc.dram_tensor("dbg_ctxT", [P, FC, Q], FP8, kind="ExternalOutput")
        dbg_x1 = nc.dram_tensor("dbg_x1", [P, 4, H], F32, kind="ExternalOutput")
        dbg_h1 = nc.dram_tensor("dbg_h1", [P, GC, Q], BF16, kind="ExternalOutput")

    wb = 1 if (use_mask or affine or debug) else 2
    with tile.TileContext(nc) as tc:
        with tc.tile_pool(name="res", bufs=1) as res, \
             tc.tile_pool(name="stream", bufs=3) as stream, \
             tc.tile_pool(name="work", bufs=2) as work, \
             tc.tile_pool(name="dram", bufs=3, space="DRAM") as dpool:

            # ---------------- resident loads (attention-critical first) ----
            qvTz_sb = res.tile([P, NH, 4, P], FP8, tag="qvTz")
            nc.sync.dma_start(qvTz_sb[:, 0], qvTz[:, 0])
            pT_sb = res.tile([P, FC, WWIN], FP8, tag="pTw")
            nc.sync.dma_start(pT_sb[:, 0], pTw[:, 0])
            kTz_h0 = stream.tile([P, 16, P], FP8, tag="kTzh", bufs=2)
            nc.sync.dma_start(kTz_h0[:], kTz[:, 0])
            vb_h0 = stream.tile([P, 8, 2, 96], FP8, tag="vbh", bufs=2)
            nc.sync.dma_start(vb_h0[:], vb2[:, 0])
            nc.sync.dma_start(qvTz_sb[:, 1:], qvTz[:, 1:])
            nc.sync.dma_start(pT_sb[:, 1:], pTw[:, 1:])
            quT_sb = res.tile([P, FC, Q], FP8, tag="quT")
            nc.sync.dma_start(quT_sb[:], quT[:])
            if use_mask:
                mb_sb = res.tile([P, 16, Q], FP8, tag="maskb")
                nc.sync.dma_start(mb_sb[:], maskb[:])
                kb_sb = res.tile([P, 16, Q], FP8, tag="keepb")
                nc.sync.dma_start(kb_sb[:], keepb[:])
            Wo_sb = res.tile([P, FC, H], FP8, tag="Wo")
            nc.sync.dma_start(Wo_sb[:], Wo[:])
            b1_sb = res.tile([P, GC], F32, tag="b1c")
            nc.sync.dma_start(b1_sb[:], b1c[:])
            if affine:
                vecs_sb = res.tile([P, 6, H], F32, tag="vecs")
                nc.sync.dma_start(vecs_sb[:], vecs[:])

            ident_raw = res.tile([P, P], F32, tag="idraw")
            make_identity(nc, ident_raw[:])
            ident = res.tile([P, P], F32R, tag="ident")
            nc.vector.tensor_copy(out=ident[:], in_=ident_raw[:])
            ident_bf = res.tile([P, P], BF16, tag="identbf")
            nc.vector.tensor_copy(out=ident_bf[:], in_=ident_raw[:])
            eps_sb = res.tile([P, 1], F32, tag="eps")
            nc.any.memset(eps_sb[:], LN_EPS)
            ones_f = res.tile([1, DK], F32, tag="onesf")
            nc.any.memset(ones_f[:], 1.0)
            ones_r = res.tile([1, DK], F32R, tag="onesr")
            nc.vector.tensor_copy(out=ones_r[:], in_=ones_f[:])

            ctxT = res.tile([P, FC, Q], FP8, tag="ctxT")

            # ---------------- attention ----------------
            with tc.tile_pool(name="ps_bd", bufs=2, space="PSUM") as ps_bd, \
                 tc.tile_pool(name="ps_st", bufs=2, space="PSUM") as ps_st, \
                 tc.tile_pool(name="ps_ctx", bufs=1, space="PSUM") as ps_ctx, \
                 tc.tile_pool(name="ps_bc", bufs=1, space="PSUM") as ps_bc:
                for h in range(NH):
                    hp, hc = DK * (h % 2), h // 2
                    if h == 0:
                        kTz_h, vb_h = kTz_h0, vb_h0
                    else:
                        kTz_h = stream.tile([P, 16, P], FP8, tag="kTzh",
                                            bufs=2)
                        nc.sync.dma_start(kTz_h[:], kTz[:, h])
                        vb_h = stream.tile([P, 8, 2, 96], FP8, tag="vbh",
                                           bufs=2)
                        nc.sync.dma_start(vb_h[:], vb2[:, h])
                    # --- stage A: dense BD rect (q part, r free) -> DRAM fp8
                    bdd = dpool.tile([4, P, BDW], FP8, tag="bdd")
                    ev = 0
                    for qt in range(4):
                        loc = 384 - 128 * qt
                        bd_sb = work.tile([P, BDW], FP8, tag="bd_sb")
                        for rc in range(5):
                            w = 512 if rc < 4 else 128
                            ps = ps_bd.tile([P, 512], F32, tag="bd")
                            nc.tensor.matmul(
                                ps[:, :w],
                                qvTz_sb[:, h, qt],
                                pT_sb[:, hc,
                                      loc + rc * 512: loc + rc * 512 + w],
                                start=True, stop=True)
                            if ev % 2 == 0:
                                nc.vector.tensor_copy(
                                    out=bd_sb[:, rc * 512: rc * 512 + w],
                                    in_=ps[:, :w])
                            else:
                                nc.scalar.activation(
                                    bd_sb[:, rc * 512: rc * 512 + w],
                                    ps[:, :w], AFT.Copy)
                            ev += 1
                        nc.sync.dma_start(bdd[qt], bd_sb[:])
                        if debug and h == 0 and qt == 0:
                            nc.sync.dma_start(dbg_rect[:], bd_sb[:])

                    # --- stage B: sheared fp8->f32 read + scores^T + exp + V
                    ctx = ps_ctx.tile([96, Q], F32, tag="ctx")
                    for jq in range(4):
                        bdsh = stream.tile(
                            [P, 4, 512], F32R, tag="bdsh",
                            bufs=2)
                        src = bass.AP(bdd.tensor, bdd.offset + 127 + 512 * jq,
                                      [[BDW - 1, P], [P * BDW, 4], [1, 512]])
                        nc.gpsimd.dma_start(bdsh[:], src)
                        if debug and h == 0 and jq == 0:
                            bdsh_c = work.tile([P, 4, 512], F32, tag="bdshc")
                            nc.vector.tensor_copy(out=bdsh_c[:],
                                                  in_=bdsh[:].bitcast(F32))
                            nc.sync.dma_start(dbg_bdsh[:], bdsh_c[:])
                        for jph in range(2):
                            jp = 2 * jq + jph
                            st = ps_st.tile([P, 1024], F32, tag="st")
                            for half in range(2):
                                jt = 2 * jp + half
                                co = 512 * half
                                for qt in range(4):
                                    nc.tensor.matmul(
                                        st[:, co + qt * P: co + (qt + 1) * P
                                           ].bitcast(F32R),
                                        bdsh[:, qt,
                                             jph * 256 + half * P:
                                             jph * 256 + (half + 1) * P],
                                        ident[:], is_transpose=True,
                                        start=(qt == 0), stop=False)
                                nc.tensor.matmul(
                                    st[:, co:co + 512],
                                    kTz_h[:, jt],
                                    quT_sb[:, hc, :],
                                    start=False, stop=True)
                                if use_mask:
                                    nc.vector.tensor_tensor(
                                        st[:, co:co + 512], st[:, co:co + 512],
                                        mb_sb[:, jt, :], ALU.add)
                            if debug and h == 0 and jp == 0:
                                st_c = work.tile([P, 1024], F32, tag="stc")
                                nc.vector.tensor_copy(out=st_c[:], in_=st[:])
                                nc.sync.dma_start(dbg_st[:], st_c[:])
                            e2 = work.tile([P, 2, Q], FP8, tag="e2")
                            nc.scalar.activation(e2[:, 0], st[:, :512],
                                                 AFT.Exp, scale=0.125)
                            nc.scalar.activation(e2[:, 1], st[:, 512:],
                                                 AFT.Exp, scale=0.125)
                            if use_mask:
                                for half in range(2):
                                    jt = 2 * jp + half
                                    nc.vector.tensor_tensor(
                                        e2[:, half], e2[:, half],
                                        kb_sb[:, jt, :], ALU.mult)
                            if debug and h == 0 and jp == 0:
                                nc.sync.dma_start(dbg_e2[:], e2[:])
                            nc.tensor.matmul(ctx[:], vb_h[:, jp], e2[:],
                                             perf_mode=DR,
                                             start=(jp == 0), stop=(jp == 7))

                    # --- stage C: normalize by softmax denominator
                    ctx_sb = work.tile([DK + 1, Q], F32, tag="ctx_sb")
                    nc.scalar.activation(ctx_sb[:], ctx[:DK + 1], AFT.Copy)
                    den_s = work.tile([1, Q], F32, tag="den_s", bufs=wb)
                    nc.scalar.mul(den_s[:], ctx_sb[DK:DK + 1, :], 1.0 / CSC)
                    den_i = work.tile([1, Q], F32, tag="den_i", bufs=wb)
                    nc.vector.reciprocal(den_i[:], den_s[:])
                    den_r = work.tile([1, Q], F32R, tag="den_r", bufs=wb)
                    nc.vector.tensor_copy(out=den_r[:], in_=den_i[:])
                    bc = ps_bc.tile([DK, Q], F32, tag="bc")
                    nc.tensor.matmul(bc[:], ones_r[:], den_r[:],
                                     start=True, stop=True)
                    nc.vector.tensor_tensor(ctxT[hp:hp + DK, hc, :],
                                            ctx_sb[:DK, :], bc[:], ALU.mult)
                    if debug and h == 0:
                        ctx_c = work.tile([DK + 1, Q], F32, tag="ctxc")
                        nc.scalar.activation(ctx_c[:], ctx[:DK + 1], AFT.Copy)
                        nc.sync.dma_start(dbg_ctx[:], ctx_c[:])

            # ---------------- Wo + LN1 + FFN + LN2 ----------------
            if debug:
                nc.sync.dma_start(dbg_ctxT[:], ctxT[:])
            x1f = res.tile([P, 4, H], F32, tag="x1f")
            x1T = res.tile([P, FC, Q], BF16, tag="x1T")

            def layer_norm(dst, src, g_row, b_row, bf_copy=None):
                # dst = LN(src) [* g + b]; src is an f32 (P, H) SBUF AP, may alias
                s1 = work.tile([P, 1], F32, tag="s1")
                nc.vector.tensor_reduce(s1[:], src, axis=AXX, op=ALU.add)
                nm = work.tile([P, 1], F32, tag="nm")
                nc.scalar.mul(nm[:], s1[:], -1.0 / H)
                nc.vector.tensor_scalar(src, src, nm[:], None, op0=ALU.add)
                sq = work.tile([P, H], F32, tag="sq", bufs=wb)
                s2 = work.tile([P, 1], F32, tag="s2")
                nc.scalar.activation(sq[:], src, AFT.Square, accum_out=s2[:])
                lnv = work.tile([P, 1], F32, tag="lnv")
                nc.scalar.activation(lnv[:], s2[:], AFT.Ln, scale=1.0 / H,
                                     bias=eps_sb[:, 0:1])
                rstd = work.tile([P, 1], F32, tag="rstd")
                nc.scalar.activation(rstd[:], lnv[:], AFT.Exp, scale=-0.5)
                if affine:
                    nc.vector.tensor_scalar(src, src, rstd[:], None, op0=ALU.mult)
                    nc.vector.tensor_tensor(sq[:], src, g_row, ALU.mult)
                    nc.vector.tensor_tensor(dst, sq[:], b_row, ALU.add)
                else:
                    nc.vector.tensor_scalar(dst, src, rstd[:], None, op0=ALU.mult)
                if bf_copy is not None:
                    nc.vector.tensor_copy(out=bf_copy, in_=dst)

            with tc.tile_pool(name="ps_ao", bufs=2, space="PSUM") as ps_ao, \
                 tc.tile_pool(name="ps_tr", bufs=2, space="PSUM") as ps_tr:
                for qt in range(4):
                    ao = ps_ao.tile([P, H], F32, tag="ao")
                    for hcp in range(FC // 2):
                        for c0, c1 in ((0, 512), (512, 768)):
                            nc.tensor.matmul(
                                ao[:, c0:c1],
                                ctxT[:, 2 * hcp:2 * hcp + 2,
                                     qt * P:(qt + 1) * P],
                                Wo_sb[:, 2 * hcp:2 * hcp + 2, c0:c1],
                                perf_mode=DR,
                                start=(hcp == 0), stop=(hcp == FC // 2 - 1))
                    ao_s = work.tile([P, H], F32, tag="ao_s", bufs=wb)
                    nc.scalar.activation(ao_s[:], ao[:], AFT.Copy,
                                         scale=1.0 / (WSC * CSC))
                    xqt = stream.tile([P, H], F32, tag="xqt", bufs=2)
                    nc.sync.dma_start(xqt[:], xq[:, qt])
                    resid = work.tile([P, H], F32, tag="resid")
                    nc.vector.tensor_tensor(resid[:], ao_s[:], xqt[:],
                                            ALU.add)
                    if affine:
                        nc.vector.tensor_tensor(resid[:], resid[:],
                                                vecs_sb[:, 0], ALU.add)
                    x1b = work.tile([P, H], BF16, tag="x1b")
                    layer_norm(x1f[:, qt], resid[:],
                               vecs_sb[:, 2] if affine else None,
                               vecs_sb[:, 3] if affine else None,
                               bf_copy=x1b[:])
                    for fcc in range(FC):
                        tp = ps_tr.tile([P, P], BF16, tag="tr")
                        nc.tensor.matmul(tp[:], x1b[:, fcc * P:(fcc + 1) * P],
                                         ident_bf[:], is_transpose=True,
                                         start=True, stop=True)
                        nc.vector.tensor_copy(
                            out=x1T[:, fcc, qt * P:(qt + 1) * P], in_=tp[:])

            if debug:
                nc.sync.dma_start(dbg_x1[:], x1f[:])
            # FFN1 + FFN2 interleaved: FFN2 accumulates into SBUF per
            # 6-gc chunk so its PSUM pool coexists with FFN1's.
            h1_tiles = [res.tile([P, Q], BF16, tag=f"h1_{gc}",
                                 name=f"h1_{gc}")
                        for gc in range(GC)]
            o_acc = [res.tile([P, H], F32, tag=f"oa{qt}",
                              name=f"oa{qt}") for qt in range(4)]
            CH = 6
            with tc.tile_pool(name="ps_h1", bufs=2, space="PSUM") as ps_h1, \
                 tc.tile_pool(name="ps_o2", bufs=2, space="PSUM") as ps_o2:
                for cc in range(GC // CH):
                    w2c = stream.tile([P, CH, H], BF16, tag="w2c", bufs=wb)
                    nc.sync.dma_start(w2c[:], W2[:, cc * CH:(cc + 1) * CH, :])
                    for gi in range(CH):
                        gc = cc * CH + gi
                        w1s = stream.tile([P, FC, P], BF16, tag="w1s",
                                          bufs=2)
                        nc.sync.dma_start(
                            w1s[:], W1[:, :, gc * P:(gc + 1) * P])
                        hp1 = ps_h1.tile([P, Q], F32, tag="h1")
                        for fc in range(FC):
                            nc.tensor.matmul(hp1[:], w1s[:, fc],
                                             x1T[:, fc, :],
                                             start=(fc == 0),
                                             stop=(fc == FC - 1))
                        nc.scalar.activation(h1_tiles[gc][:], hp1[:],
                                             AFT.Gelu,
                                             bias=b1_sb[:, gc:gc + 1])
                    for qt in range(4):
                        po = ps_o2.tile([P, H], F32, tag="po")
                        for gi in range(CH):
                            gc = cc * CH + gi
                            nc.tensor.matmul(
                                po[:, :512],
                                h1_tiles[gc][:, qt * P:(qt + 1) * P],
                                w2c[:, gi, :512],
                                start=(gi == 0), stop=(gi == CH - 1))
                            nc.tensor.matmul(
                                po[:, 512:],
                                h1_tiles[gc][:, qt * P:(qt + 1) * P],
                                w2c[:, gi, 512:],
                                start=(gi == 0), stop=(gi == CH - 1))
                        if cc == 0:
                            nc.vector.tensor_copy(out=o_acc[qt][:],
                                                  in_=po[:])
                        else:
                            nc.vector.tensor_tensor(
                                o_acc[qt][:], o_acc[qt][:], po[:],
                                ALU.add)

            if debug:
                for gc in range(GC):
                    nc.sync.dma_start(dbg_h1[:, gc], h1_tiles[gc][:])
            # LN2
            if True:
                for qt in range(4):
                    r2 = work.tile([P, H], F32, tag="resid2", bufs=wb)
                    nc.vector.tensor_tensor(r2[:], o_acc[qt][:], x1f[:, qt],
                                            ALU.add)
                    if affine:
                        nc.vector.tensor_tensor(r2[:], r2[:],
                                                vecs_sb[:, 1], ALU.add)
                    o_sb = work.tile([P, H], F32, tag="osb")
                    layer_norm(o_sb[:], r2[:],
                               vecs_sb[:, 4] if affine else None,
                               vecs_sb[:, 5] if affine else None)
                    nc.sync.dma_start(out[:, qt], o_sb[:])
    nc.compile()
    return nc


# --------------------------------------------------------------------------
def _chunk_pf(w):
    """(768, X) -> (128, 6, X) with row d' = 128*chunk + partition."""
    return np.ascontiguousarray(w.reshape(FC, P, -1).transpose(1, 0, 2))


def kernel(hidden_states, attention_mask, pos_emb,
           Wq, bq, Wk, bk, Wv, bv, Wp, pos_bias_u, pos_bias_v, Wo, bo,
           ln1_g, ln1_b, W1, b1, W2, b2, ln2_g, ln2_b):
    f32 = lambda x: np.asarray(x, dtype=np.float32)
    hidden_states = f32(hidden_states)
    pos_emb = f32(pos_emb)
    mask = np.asarray(attention_mask)
    use_mask = bool(mask.any())
    affine = not (np.all(f32(ln1_g) == 1) and np.all(f32(ln1_b) == 0)
                  and np.all(f32(ln2_g) == 1) and np.all(f32(ln2_b) == 0)
                  and np.all(f32(bo) == 0) and np.all(f32(b2) == 0))

    debug = bool(os.environ.get("BERT_KERNEL_DEBUG"))
    if "d1" not in _cache:
        _cache["d1"] = _build_d1()
    key = ("d2", use_mask, affine, debug)
    if key not in _cache:
        _cache[key] = _build_d2(use_mask, affine, debug)
    d1, d2 = _cache["d1"], _cache[key]

    hf = hidden_states.reshape(B * T, H)
    xT_full = _chunk_pf(np.ascontiguousarray(hf.T)).astype(F8)
    posT_pad = np.zeros((H, 4096), np.float32)
    posT_pad[:, :2 * T - 1] = pos_emb[0].T
    posT_full = _chunk_pf(posT_pad).astype(F8)

    def _w_d1(w):
        # [p, dc, fcp, t, m] = WSC * w[256*fcp + 128*t + p, 128*dc + m]
        return np.ascontiguousarray(
            (f32(w) * WSC).reshape(3, 2, P, FC, P).transpose(2, 3, 0, 1, 4)
        ).astype(F8)

    wq_c, wk_c, wv_c, wp_c = map(_w_d1, (Wq, Wk, Wv, Wp))
    bq_c = f32(bq).reshape(FC, P).T.copy()
    bk_c = f32(bk).reshape(FC, P).T.copy()
    bv_c = f32(bv).reshape(FC, P).T.copy()

    in1 = []
    for c in range(NCORE):
        sl = slice(512 * c, 512 * c + 512)
        in1.append({
            "xT": np.ascontiguousarray(xT_full[:, :, sl]),
            "posT": np.ascontiguousarray(posT_full[:, :, sl]),
            "Wq": wq_c, "Wk": wk_c, "Wv": wv_c, "Wp": wp_c,
            "bq": bq_c, "bk": bk_c, "bv": bv_c,
        })
    _trace = bool(os.environ.get("BERT_KERNEL_TRACE"))
    _res1 = run_bass_kernel_spmd(d1, in1, core_ids=list(range(NCORE)),
                                 trace=_trace)
    PROFILE["d1_ns"] = _res1.exec_time_ns
    PROFILE["d1_res"] = _res1
    r1 = _res1.results

    qT_full = np.concatenate([r["qT"] for r in r1], axis=2).astype(np.float32)
    kT_full = np.concatenate([r["kT"] for r in r1], axis=2).astype(np.float32)
    vT_full = np.concatenate([r["vT"] for r in r1], axis=2).astype(np.float32)
    pT_full = np.concatenate([r["pT"] for r in r1], axis=2).astype(np.float32)
    pT_full[:, :, 2 * T - 1:] = 0

    pbu_c = f32(pos_bias_u).reshape(NH * DK).reshape(FC, P).T.copy()
    pbv_c = f32(pos_bias_v).reshape(NH * DK).reshape(FC, P).T.copy()
    quT_full = (qT_full + pbu_c[:, :, None]).astype(F8)
    qvT_full = qT_full + pbv_c[:, :, None]                    # f32
    pT_f8 = pT_full.astype(F8)

    wo_c = (_chunk_pf(f32(Wo)) * WSC).astype(F8)
    w1_c = _chunk_pf(f32(W1)).astype(BF)
    w2_c = np.ascontiguousarray(
        f32(W2).reshape(GC, P, H).transpose(1, 0, 2)).astype(BF)
    b1_c = f32(b1).reshape(GC, P).T.copy()
    if affine:
        vecs = np.stack([np.broadcast_to(f32(x), (P, H)) for x in
                         (bo, b2, ln1_g, ln1_b, ln2_g, ln2_b)], axis=1).copy()

    in2 = []
    for c in range(NCORE):
        b_ = c // 4
        q0 = 512 * (c % 4)
        w0 = 1536 - q0
        tsl = slice(T * b_, T * b_ + T)

        # zero-padded K=128 stationaries (fp8), partition dim first
        qvTz = np.zeros((P, NH, 4, P), F8)
        kTz = np.zeros((P, NH, 16, P), F8)
        for h in range(NH):
            hp, hcc = DK * (h % 2), h // 2
            qvTz[hp:hp + DK, h] = qvT_full[
                hp:hp + DK, hcc, 512 * c:512 * c + 512
            ].reshape(DK, 4, P).astype(F8)
            kTz[hp:hp + DK, h] = kT_full[
                hp:hp + DK, hcc, tsl].reshape(DK, 16, P).astype(F8)

        vv = vT_full[:, :, tsl]                                   # (128,6,2048)
        vmat = np.ascontiguousarray(
            vv.transpose(1, 0, 2).reshape(H, T))                  # (768,2048)=v.T
        arr = vmat.reshape(NH, DK, 16, P).transpose(0, 3, 2, 1)   # (12,128,16,64)
        # vb2[p, h, jp, t, d] = arr[h, p, 2*jp + t, d]; ones column at d=64
        vb_c = np.zeros((P, NH, 8, 2, 96), F8)
        vb_c[:, :, :, :, :DK] = arr.reshape(NH, P, 8, 2, DK).transpose(
            1, 0, 2, 3, 4).astype(F8)
        vb_c[:, :, :, :, DK] = 1.0
        entry = {
            "qvTz": qvTz,
            "pTw": np.ascontiguousarray(pT_f8[:, :, w0:w0 + WWIN]),
            "quT": np.ascontiguousarray(quT_full[:, :, 512 * c:512 * c + 512]),
            "kTz": kTz,
            "vb2": vb_c,
            "xq": np.ascontiguousarray(
                hf[T * b_ + q0: T * b_ + q0 + 512].reshape(4, P, H)
                .transpose(1, 0, 2)),
            "Wo": wo_c, "W1": w1_c, "W2": w2_c, "b1c": b1_c,
        }
        if affine:
            entry["vecs"] = vecs
        if use_mask:
            m = f32(mask[b_])
            mT = m.T[:, q0:q0 + 512]                              # (2048,512) j,q
            entry["maskb"] = (mT.reshape(16, P, 512).transpose(1, 0, 2)
                              * np.float32(-240.0)).astype(F8)
            entry["keepb"] = (1.0 - mT.reshape(16, P, 512)
                              .transpose(1, 0, 2)).astype(F8)
        in2.append(entry)

    PROFILE["in2"] = in2
    _res2 = run_bass_kernel_spmd(d2, in2, core_ids=list(range(NCORE)),
                                 trace=_trace)
    PROFILE["d2_ns"] = _res2.exec_time_ns
    PROFILE["d2_res"] = _res2
    r2 = _res2.results

    outp = np.zeros((B, T, H), np.float32)
    for c in range(NCORE):
        b_ = c // 4
        q0 = 512 * (c % 4)
        outp[b_, q0:q0 + 512] = r2[c]["out"].transpose(1, 0, 2).reshape(512, H)
    return outp


# revision 42
# speedup vs baseline: 1.1878x; 1.0255x over previous
"""Trainium2 Bass kernel for a Transformer-XL style BertLayer (relative attention).

Sharding (8 NeuronCores, full inputs in / full output out):
  Dispatch 1: token-sharded transposed projections qT/kT/vT/pT in bf16.
  Host: reassemble; add pos_bias_u/v; build fp8 operands (zero-padded
    K=128 stationaries for BD/AC, DoubleRow-paired V / W1 / W2 / Wo);
    query-split for dispatch 2 (core c: batch c//4, queries
    [512*(c%4), +512)).
  Dispatch 2: attention with keys-on-partitions. The rel-shift is done by
    writing the dense BD position-score matrix to DRAM (rect, fp8) and
    reading it back through a sheared flat access pattern with an
    fp8->f32 casting SWDGE DMA, then PE-transposing into the scores^T
    PSUM accumulation on top of the content scores. Softmax denominators
    ride as a ones-column appended to V (fp8 DoubleRow matmul). Then
    Wo (fp8 DoubleRow) + residual/LN1 + FFN in fp8 DoubleRow (exact
    GELU) + residual/LN2.
"""

import os
import sys
import numpy as np
import ml_dtypes

sys.path.insert(0, "/opt/trn_rl_repo")

import concourse.bass as bass
import concourse.mybir as mybir
import concourse.tile as tile
from concourse import bacc
from concourse.bass_utils import run_bass_kernel_spmd
from concourse.masks import make_identity

BF = ml_dtypes.bfloat16
F8 = ml_dtypes.float8_e4m3
F32, BF16, F32R = mybir.dt.float32, mybir.dt.bfloat16, mybir.dt.float32r
FP8 = mybir.dt.float8e4
DR = mybir.MatmulPerfMode.DoubleRow
AFT = mybir.ActivationFunctionType
ALU = mybir.AluOpType
AXX = mybir.AxisListType.X

B, T, H, NH, DK = 2, 2048, 768, 12, 64
P = 128
FC = H // P            # 6 feature chunks
GC = 3072 // P         # 24 intermediate chunks
Q = 512                # queries per core
NCORE = 8
WWIN = 2560            # pT window width per core
BDW = 2176             # BD rect row width (2175 used + 1 pad)
LN_EPS = 1e-5
WSC = 16.0             # fp8 weight pre-scale (host side)
CSC = 32.0             # ctxT pre-scale

_cache = {}
PROFILE = {}


def _build_d1():
    nc = bacc.Bacc(None, target_bir_lowering=False)
    xT = nc.dram_tensor("xT", [P, FC, Q], FP8, kind="ExternalInput")
    posT = nc.dram_tensor("posT", [P, FC, Q], FP8, kind="ExternalInput")
    ws = {n: nc.dram_tensor(n, [P, FC, 3, 2, P], FP8, kind="ExternalInput")
          for n in ("Wq", "Wk", "Wv", "Wp")}
    bs = {n: nc.dram_tensor(n, [P, FC], F32, kind="ExternalInput")
          for n in ("bq", "bk", "bv")}
    outs = {n: nc.dram_tensor(n, [P, FC, Q], BF16, kind="ExternalOutput")
            for n in ("qT", "kT", "vT", "pT")}

    with tile.TileContext(nc) as tc:
        with tc.tile_pool(name="sb", bufs=2) as sb, \
             tc.tile_pool(name="wp", bufs=2) as wp, \
             tc.tile_pool(name="ps", bufs=3, space="PSUM") as psp:
            xT_sb = sb.tile([P, FC, Q], FP8, tag="x")
            nc.sync.dma_start(xT_sb[:], xT[:])
            posT_sb = sb.tile([P, FC, Q], FP8, tag="p")
            nc.scalar.dma_start(posT_sb[:], posT[:])
            bias_sb = {}
            for n in bs:
                t = sb.tile([P, FC], F32, tag=n)
                nc.scalar.dma_start(t[:], bs[n][:])
                bias_sb[n] = t

            ev = 0
            for wn, bn, on, src in (("Wq", "bq", "qT", xT_sb),
                                    ("Wk", "bk", "kT", xT_sb),
                                    ("Wv", "bv", "vT", xT_sb),
                                    ("Wp", None, "pT", posT_sb)):
                w_sb = wp.tile([P, FC, 3, 2, P], FP8, tag="w")
                (nc.sync if ev % 2 == 0 else nc.scalar).dma_start(
                    w_sb[:], ws[wn][:])
                ev += 1
                o_sb = sb.tile([P, FC, Q], BF16, tag="o")
                for dc in range(FC):
                    ps = psp.tile([P, Q], F32, tag="ps")
                    for fcp in range(3):
                        nc.tensor.matmul(ps[:], w_sb[:, dc, fcp],
                                         src[:, 2 * fcp:2 * fcp + 2, :],
                                         perf_mode=DR,
                                         start=(fcp == 0), stop=(fcp == 2))
                    if bn is None:
                        nc.scalar.activation(o_sb[:, dc], ps[:], AFT.Copy,
                                             scale=1.0 / WSC)
                    else:
                        nc.scalar.activation(o_sb[:, dc], ps[:], AFT.Identity,
                                             scale=1.0 / WSC,
                                             bias=bias_sb[bn][:, dc:dc + 1])
                nc.sync.dma_start(outs[on][:], o_sb[:])
    nc.compile()
    return nc


def _build_d2(use_mask: bool, affine: bool, debug: bool = False):
    nc = bacc.Bacc(None, target_bir_lowering=False)
    qvTz = nc.dram_tensor("qvTz", [P, NH, 4, P], FP8, kind="ExternalInput")
    pTw = nc.dram_tensor("pTw", [P, FC, WWIN], FP8, kind="ExternalInput")
    quT = nc.dram_tensor("quT", [P, FC, Q], FP8, kind="ExternalInput")
    kTz = nc.dram_tensor("kTz", [P, NH, 16, P], FP8, kind="ExternalInput")
    vb2 = nc.dram_tensor("vb2", [P, NH, 8, 2, 96], FP8, kind="ExternalInput")
    Wo = nc.dram_tensor("Wo", [P, FC, H], FP8, kind="ExternalInput")
    W1 = nc.dram_tensor("W1", [P, FC, 3072], BF16, kind="ExternalInput")
    W2 = nc.dram_tensor("W2", [P, GC, H], BF16, kind="ExternalInput")
    b1c = nc.dram_tensor("b1c", [P, GC], F32, kind="ExternalInput")
    xq = nc.dram_tensor("xq", [P, 4, H], F32, kind="ExternalInput")
    if affine:
        # rows: 0=bo 1=b2 2=ln1_g 3=ln1_b 4=ln2_g 5=ln2_b (replicated over partitions)
        vecs = nc.dram_tensor("vecs", [P, 6, H], F32, kind="ExternalInput")
    if use_mask:
        maskb = nc.dram_tensor("maskb", [P, 16, Q], FP8, kind="ExternalInput")
        keepb = nc.dram_tensor("keepb", [P, 16, Q], FP8, kind="ExternalInput")
    out = nc.dram_tensor("out", [P, 4, H], F32, kind="ExternalOutput")
    if debug:
        dbg_rect = nc.dram_tensor("dbg_rect", [P, BDW], FP8, kind="ExternalOutput")
        dbg_bdsh = nc.dram_tensor("dbg_bdsh", [P, 4, 512], F32, kind="ExternalOutput")
        dbg_st = nc.dram_tensor("dbg_st", [P, 1024], F32, kind="ExternalOutput")
        dbg_e2 = nc.dram_tensor("dbg_e2", [P, 2, Q], FP8, kind="ExternalOutput")
        dbg_ctx = nc.dram_tensor("dbg_ctx", [DK + 1, Q], F32, kind="ExternalOutput")
        dbg_ctxT = nc.dram_tensor("dbg_ctxT", [P, FC, Q], FP8, kind="ExternalOutput")
        dbg_x1 = nc.dram_tensor("dbg_x1", [P, 4, H], F32, kind="ExternalOutput")
        dbg_h1 = nc.dram_tensor("dbg_h1", [P, GC, Q], BF16, kind="ExternalOutput")

    wb = 1 if (use_mask or affine or debug) else 2
    with tile.TileContext(nc) as tc:
        with tc.tile_pool(name="res", bufs=1) as res, \
             tc.tile_pool(name="stream", bufs=3) as stream, \
             tc.tile_pool(name="work", bufs=2) as work, \
             tc.tile_pool(name="dram", bufs=4, space="DRAM") as dpool:

            # ---------------- resident loads (attention-critical first) ----
            qvTz_sb = res.tile([P, NH, 4, P], FP8, tag="qvTz")
            nc.sync.dma_start(qvTz_sb[:, 0], qvTz[:, 0])
            pT_sb = res.tile([P, FC, WWIN], FP8, tag="pTw")
            nc.sync.dma_start(pT_sb[:, 0], pTw[:, 0])
            kTz_h0 = stream.tile([P, 16, P], FP8, tag="kTzh", bufs=2)
            nc.sync.dma_start(kTz_h0[:], kTz[:, 0])
            vb_h0 = stream.tile([P, 8, 2, 96], FP8, tag="vbh", bufs=2)
            nc.sync.dma_start(vb_h0[:], vb2[:, 0])
            nc.sync.dma_start(qvTz_sb[:, 1:], qvTz[:, 1:])
            nc.sync.dma_start(pT_sb[:, 1:], pTw[:, 1:])
            quT_sb = res.tile([P, FC, Q], FP8, tag="quT")
            nc.sync.dma_start(quT_sb[:], quT[:])
            if use_mask:
                mb_sb = res.tile([P, 16, Q], FP8, tag="maskb")
                nc.sync.dma_start(mb_sb[:], maskb[:])
                kb_sb = res.tile([P, 16, Q], FP8, tag="keepb")
                nc.sync.dma_start(kb_sb[:], keepb[:])
            Wo_sb = res.tile([P, FC, H], FP8, tag="Wo")
            nc.sync.dma_start(Wo_sb[:], Wo[:])
            b1_sb = res.tile([P, GC], F32, tag="b1c")
            nc.sync.dma_start(b1_sb[:], b1c[:])
            if affine:
                vecs_sb = res.tile([P, 6, H], F32, tag="vecs")
                nc.sync.dma_start(vecs_sb[:], vecs[:])

            ident_raw = res.tile([P, P], F32, tag="idraw")
            make_identity(nc, ident_raw[:])
            ident = res.tile([P, P], F32R, tag="ident")
            nc.vector.tensor_copy(out=ident[:], in_=ident_raw[:])
            ident_bf = res.tile([P, P], BF16, tag="identbf")
            nc.vector.tensor_copy(out=ident_bf[:], in_=ident_raw[:])
            eps_sb = res.tile([P, 1], F32, tag="eps")
            nc.any.memset(eps_sb[:], LN_EPS)
            ones_f = res.tile([1, DK], F32, tag="onesf")
            nc.any.memset(ones_f[:], 1.0)
            ones_r = res.tile([1, DK], F32R, tag="onesr")
            nc.vector.tensor_copy(out=ones_r[:], in_=ones_f[:])

            ctxT = res.tile([P, FC, Q], FP8, tag="ctxT")

            # ---------------- attention ----------------
            with tc.tile_pool(name="ps_bd", bufs=2, space="PSUM") as ps_bd, \
                 tc.tile_pool(name="ps_st", bufs=2, space="PSUM") as ps_st, \
                 tc.tile_pool(name="ps_ctx", bufs=1, space="PSUM") as ps_ctx, \
                 tc.tile_pool(name="ps_bc", bufs=1, space="PSUM") as ps_bc:
                for h in range(NH):
                    hp, hc = DK * (h % 2), h // 2
                    if h == 0:
                        kTz_h, vb_h = kTz_h0, vb_h0
                    else:
                        kTz_h = stream.tile([P, 16, P], FP8, tag="kTzh",
                                            bufs=2)
                        nc.sync.dma_start(kTz_h[:], kTz[:, h])
                        vb_h = stream.tile([P, 8, 2, 96], FP8, tag="vbh",
                                           bufs=2)
                        nc.sync.dma_start(vb_h[:], vb2[:, h])
                    # --- stage A: dense BD rect (q part, r free) -> DRAM fp8
                    bdd = dpool.tile([4, P, BDW], FP8, tag="bdd")
                    ev = 0
                    for qt in range(4):
                        loc = 384 - 128 * qt
                        bd_sb = work.tile([P, BDW], FP8, tag="bd_sb",
                                          bufs=2 + (wb == 2))
                        for rc in range(5):
                            w = 512 if rc < 4 else 128
                            ps = ps_bd.tile([P, 512], F32, tag="bd")
                            nc.tensor.matmul(
                                ps[:, :w],
                                qvTz_sb[:, h, qt],
                                pT_sb[:, hc,
                                      loc + rc * 512: loc + rc * 512 + w],
                                start=True, stop=True)
                            if ev % 2 == 0:
                                nc.vector.tensor_copy(
                                    out=bd_sb[:, rc * 512: rc * 512 + w],
                                    in_=ps[:, :w])
                            else:
                                nc.scalar.activation(
                                    bd_sb[:, rc * 512: rc * 512 + w],
                                    ps[:, :w], AFT.Copy)
                            ev += 1
                        nc.sync.dma_start(bdd[qt], bd_sb[:])
                        if debug and h == 0 and qt == 0:
                            nc.sync.dma_start(dbg_rect[:], bd_sb[:])

                    # --- stage B: sheared fp8->f32 read + scores^T + exp + V
                    ctx = ps_ctx.tile([96, Q], F32, tag="ctx")
                    for jq in range(4):
                        bdsh = stream.tile(
                            [P, 4, 512], F32R, tag="bdsh",
                            bufs=2)
                        src = bass.AP(bdd.tensor, bdd.offset + 127 + 512 * jq,
                                      [[BDW - 1, P], [P * BDW, 4], [1, 512]])
                        nc.gpsimd.dma_start(bdsh[:], src)
                        if debug and h == 0 and jq == 0:
                            bdsh_c = work.tile([P, 4, 512], F32, tag="bdshc")
                            nc.vector.tensor_copy(out=bdsh_c[:],
                                                  in_=bdsh[:].bitcast(F32))
                            nc.sync.dma_start(dbg_bdsh[:], bdsh_c[:])
                        for jph in range(2):
                            jp = 2 * jq + jph
                            st = ps_st.tile([P, 1024], F32, tag="st")
                            for half in range(2):
                                jt = 2 * jp + half
                                co = 512 * half
                                for qt in range(4):
                                    nc.tensor.matmul(
                                        st[:, co + qt * P: co + (qt + 1) * P
                                           ].bitcast(F32R),
                                        bdsh[:, qt,
                                             jph * 256 + half * P:
                                             jph * 256 + (half + 1) * P],
                                        ident[:], is_transpose=True,
                                        start=(qt == 0), stop=False)
                                nc.tensor.matmul(
                                    st[:, co:co + 512],
                                    kTz_h[:, jt],
                                    quT_sb[:, hc, :],
                                    start=False, stop=True)
                                if use_mask:
                                    nc.vector.tensor_tensor(
                                        st[:, co:co + 512], st[:, co:co + 512],
                                        mb_sb[:, jt, :], ALU.add)
                            if debug and h == 0 and jp == 0:
                                st_c = work.tile([P, 1024], F32, tag="stc")
                                nc.vector.tensor_copy(out=st_c[:], in_=st[:])
                                nc.sync.dma_start(dbg_st[:], st_c[:])
                            e2 = work.tile([P, 2, Q], FP8, tag="e2",
                                           bufs=2 + (wb == 2))
                            nc.scalar.activation(e2[:, 0], st[:, :512],
                                                 AFT.Exp, scale=0.125)
                            nc.scalar.activation(e2[:, 1], st[:, 512:],
                                                 AFT.Exp, scale=0.125)
                            if use_mask:
                                for half in range(2):
                                    jt = 2 * jp + half
                                    nc.vector.tensor_tensor(
                                        e2[:, half], e2[:, half],
                                        kb_sb[:, jt, :], ALU.mult)
                            if debug and h == 0 and jp == 0:
                                nc.sync.dma_start(dbg_e2[:], e2[:])
                            nc.tensor.matmul(ctx[:], vb_h[:, jp], e2[:],
                                             perf_mode=DR,
                                             start=(jp == 0), stop=(jp == 7))

                    # --- stage C: normalize by softmax denominator
                    ctx_sb = work.tile([DK + 1, Q], F32, tag="ctx_sb")
                    nc.scalar.activation(ctx_sb[:], ctx[:DK + 1], AFT.Copy)
                    den_s = work.tile([1, Q], F32, tag="den_s", bufs=wb)
                    nc.scalar.mul(den_s[:], ctx_sb[DK:DK + 1, :], 1.0 / CSC)
                    den_i = work.tile([1, Q], F32, tag="den_i", bufs=wb)
                    nc.vector.reciprocal(den_i[:], den_s[:])
                    den_r = work.tile([1, Q], F32R, tag="den_r", bufs=wb)
                    nc.vector.tensor_copy(out=den_r[:], in_=den_i[:])
                    bc = ps_bc.tile([DK, Q], F32, tag="bc")
                    nc.tensor.matmul(bc[:], ones_r[:], den_r[:],
                                     start=True, stop=True)
                    nc.vector.tensor_tensor(ctxT[hp:hp + DK, hc, :],
                                            ctx_sb[:DK, :], bc[:], ALU.mult)
                    if debug and h == 0:
                        ctx_c = work.tile([DK + 1, Q], F32, tag="ctxc")
                        nc.scalar.activation(ctx_c[:], ctx[:DK + 1], AFT.Copy)
                        nc.sync.dma_start(dbg_ctx[:], ctx_c[:])

            # ---------------- Wo + LN1 + FFN + LN2 ----------------
            if debug:
                nc.sync.dma_start(dbg_ctxT[:], ctxT[:])
            x1f = res.tile([P, 4, H], F32, tag="x1f")
            x1T = res.tile([P, FC, Q], BF16, tag="x1T")

            def layer_norm(dst, src, g_row, b_row, bf_copy=None):
                # dst = LN(src) [* g + b]; src is an f32 (P, H) SBUF AP, may alias
                s1 = work.tile([P, 1], F32, tag="s1")
                nc.vector.tensor_reduce(s1[:], src, axis=AXX, op=ALU.add)
                nm = work.tile([P, 1], F32, tag="nm")
                nc.scalar.mul(nm[:], s1[:], -1.0 / H)
                nc.vector.tensor_scalar(src, src, nm[:], None, op0=ALU.add)
                sq = work.tile([P, H], F32, tag="sq", bufs=wb)
                s2 = work.tile([P, 1], F32, tag="s2")
                nc.scalar.activation(sq[:], src, AFT.Square, accum_out=s2[:])
                lnv = work.tile([P, 1], F32, tag="lnv")
                nc.scalar.activation(lnv[:], s2[:], AFT.Ln, scale=1.0 / H,
                                     bias=eps_sb[:, 0:1])
                rstd = work.tile([P, 1], F32, tag="rstd")
                nc.scalar.activation(rstd[:], lnv[:], AFT.Exp, scale=-0.5)
                if affine:
                    nc.vector.tensor_scalar(src, src, rstd[:], None, op0=ALU.mult)
                    nc.vector.tensor_tensor(sq[:], src, g_row, ALU.mult)
                    nc.vector.tensor_tensor(dst, sq[:], b_row, ALU.add)
                else:
                    nc.vector.tensor_scalar(dst, src, rstd[:], None, op0=ALU.mult)
                if bf_copy is not None:
                    nc.vector.tensor_copy(out=bf_copy, in_=dst)

            with tc.tile_pool(name="ps_ao", bufs=2, space="PSUM") as ps_ao, \
                 tc.tile_pool(name="ps_tr", bufs=2, space="PSUM") as ps_tr:
                for qt in range(4):
                    ao = ps_ao.tile([P, H], F32, tag="ao")
                    for hcp in range(FC // 2):
                        for c0, c1 in ((0, 512), (512, 768)):
                            nc.tensor.matmul(
                                ao[:, c0:c1],
                                ctxT[:, 2 * hcp:2 * hcp + 2,
                                     qt * P:(qt + 1) * P],
                                Wo_sb[:, 2 * hcp:2 * hcp + 2, c0:c1],
                                perf_mode=DR,
                                start=(hcp == 0), stop=(hcp == FC // 2 - 1))
                    ao_s = work.tile([P, H], F32, tag="ao_s", bufs=wb)
                    nc.scalar.activation(ao_s[:], ao[:], AFT.Copy,
                                         scale=1.0 / (WSC * CSC))
                    xqt = stream.tile([P, H], F32, tag="xqt", bufs=2)
                    nc.sync.dma_start(xqt[:], xq[:, qt])
                    resid = work.tile([P, H], F32, tag="resid")
                    nc.vector.tensor_tensor(resid[:], ao_s[:], xqt[:],
                                            ALU.add)
                    if affine:
                        nc.vector.tensor_tensor(resid[:], resid[:],
                                                vecs_sb[:, 0], ALU.add)
                    x1b = work.tile([P, H], BF16, tag="x1b")
                    layer_norm(x1f[:, qt], resid[:],
                               vecs_sb[:, 2] if affine else None,
                               vecs_sb[:, 3] if affine else None,
                               bf_copy=x1b[:])
                    for fcc in range(FC):
                        tp = ps_tr.tile([P, P], BF16, tag="tr")
                        nc.tensor.matmul(tp[:], x1b[:, fcc * P:(fcc + 1) * P],
                                         ident_bf[:], is_transpose=True,
                                         start=True, stop=True)
                        nc.vector.tensor_copy(
                            out=x1T[:, fcc, qt * P:(qt + 1) * P], in_=tp[:])

            if debug:
                nc.sync.dma_start(dbg_x1[:], x1f[:])
            # FFN1 + FFN2 interleaved: FFN2 accumulates into SBUF per
            # 6-gc chunk so its PSUM pool coexists with FFN1's.
            h1_tiles = [res.tile([P, Q], BF16, tag=f"h1_{gc}",
                                 name=f"h1_{gc}")
                        for gc in range(GC)]
            o_acc = [res.tile([P, H], F32, tag=f"oa{qt}",
                              name=f"oa{qt}") for qt in range(4)]
            CH = 6
            with tc.tile_pool(name="ps_h1", bufs=2, space="PSUM") as ps_h1, \
                 tc.tile_pool(name="ps_o2", bufs=2, space="PSUM") as ps_o2:
                for cc in range(GC // CH):
                    w2c = stream.tile([P, CH, H], BF16, tag="w2c", bufs=wb)
                    nc.sync.dma_start(w2c[:], W2[:, cc * CH:(cc + 1) * CH, :])
                    for gi in range(CH):
                        gc = cc * CH + gi
                        w1s = stream.tile([P, FC, P], BF16, tag="w1s",
                                          bufs=2)
                        nc.sync.dma_start(
                            w1s[:], W1[:, :, gc * P:(gc + 1) * P])
                        hp1 = ps_h1.tile([P, Q], F32, tag="h1")
                        for fc in range(FC):
                            nc.tensor.matmul(hp1[:], w1s[:, fc],
                                             x1T[:, fc, :],
                                             start=(fc == 0),
                                             stop=(fc == FC - 1))
                        nc.scalar.activation(h1_tiles[gc][:], hp1[:],
                                             AFT.Gelu,
                                             bias=b1_sb[:, gc:gc + 1])
                    for qt in range(4):
                        po = ps_o2.tile([P, H], F32, tag="po")
                        for gi in range(CH):
                            gc = cc * CH + gi
                            nc.tensor.matmul(
                                po[:, :512],
                                h1_tiles[gc][:, qt * P:(qt + 1) * P],
                                w2c[:, gi, :512],
                                start=(gi == 0), stop=(gi == CH - 1))
                            nc.tensor.matmul(
                                po[:, 512:],
                                h1_tiles[gc][:, qt * P:(qt + 1) * P],
                                w2c[:, gi, 512:],
                                start=(gi == 0), stop=(gi == CH - 1))
                        if cc == 0:
                            nc.vector.tensor_copy(out=o_acc[qt][:],
                                                  in_=po[:])
                        else:
                            nc.vector.tensor_tensor(
                                o_acc[qt][:], o_acc[qt][:], po[:],
                                ALU.add)

            if debug:
                for gc in range(GC):
                    nc.sync.dma_start(dbg_h1[:, gc], h1_tiles[gc][:])
            # LN2
            if True:
                for qt in range(4):
                    r2 = work.tile([P, H], F32, tag="resid2", bufs=wb)
                    nc.vector.tensor_tensor(r2[:], o_acc[qt][:], x1f[:, qt],
                                            ALU.add)
                    if affine:
                        nc.vector.tensor_tensor(r2[:], r2[:],
                                                vecs_sb[:, 1], ALU.add)
                    o_sb = work.tile([P, H], F32, tag="osb")
                    layer_norm(o_sb[:], r2[:],
                               vecs_sb[:, 4] if affine else None,
                               vecs_sb[:, 5] if affine else None)
                    nc.sync.dma_start(out[:, qt], o_sb[:])
    nc.compile()
    return nc


# --------------------------------------------------------------------------
def _chunk_pf(w):
    """(768, X) -> (128, 6, X) with row d' = 128*chunk + partition."""
    return np.ascontiguousarray(w.reshape(FC, P, -1).transpose(1, 0, 2))


def kernel(hidden_states, attention_mask, pos_emb,
           Wq, bq, Wk, bk, Wv, bv, Wp, pos_bias_u, pos_bias_v, Wo, bo,
           ln1_g, ln1_b, W1, b1, W2, b2, ln2_g, ln2_b):
    f32 = lambda x: np.asarray(x, dtype=np.float32)
    hidden_states = f32(hidden_states)
    pos_emb = f32(pos_emb)
    mask = np.asarray(attention_mask)
    use_mask = bool(mask.any())
    affine = not (np.all(f32(ln1_g) == 1) and np.all(f32(ln1_b) == 0)
                  and np.all(f32(ln2_g) == 1) and np.all(f32(ln2_b) == 0)
                  and np.all(f32(bo) == 0) and np.all(f32(b2) == 0))

    debug = bool(os.environ.get("BERT_KERNEL_DEBUG"))
    if "d1" not in _cache:
        _cache["d1"] = _build_d1()
    key = ("d2", use_mask, affine, debug)
    if key not in _cache:
        _cache[key] = _build_d2(use_mask, affine, debug)
    d1, d2 = _cache["d1"], _cache[key]

    hf = hidden_states.reshape(B * T, H)
    xT_full = _chunk_pf(np.ascontiguousarray(hf.T)).astype(F8)
    posT_pad = np.zeros((H, 4096), np.float32)
    posT_pad[:, :2 * T - 1] = pos_emb[0].T
    posT_full = _chunk_pf(posT_pad).astype(F8)

    def _w_d1(w):
        # [p, dc, fcp, t, m] = WSC * w[256*fcp + 128*t + p, 128*dc + m]
        return np.ascontiguousarray(
            (f32(w) * WSC).reshape(3, 2, P, FC, P).transpose(2, 3, 0, 1, 4)
        ).astype(F8)

    wq_c, wk_c, wv_c, wp_c = map(_w_d1, (Wq, Wk, Wv, Wp))
    bq_c = f32(bq).reshape(FC, P).T.copy()
    bk_c = f32(bk).reshape(FC, P).T.copy()
    bv_c = f32(bv).reshape(FC, P).T.copy()

    in1 = []
    for c in range(NCORE):
        sl = slice(512 * c, 512 * c + 512)
        in1.append({
            "xT": np.ascontiguousarray(xT_full[:, :, sl]),
            "posT": np.ascontiguousarray(posT_full[:, :, sl]),
            "Wq": wq_c, "Wk": wk_c, "Wv": wv_c, "Wp": wp_c,
            "bq": bq_c, "bk": bk_c, "bv": bv_c,
        })
    _trace = bool(os.environ.get("BERT_KERNEL_TRACE"))
    _res1 = run_bass_kernel_spmd(d1, in1, core_ids=list(range(NCORE)),
                                 trace=_trace)
    PROFILE["d1_ns"] = _res1.exec_time_ns
    PROFILE["d1_res"] = _res1
    r1 = _res1.results

    qT_full = np.concatenate([r["qT"] for r in r1], axis=2).astype(np.float32)
    kT_full = np.concatenate([r["kT"] for r in r1], axis=2).astype(np.float32)
    vT_full = np.concatenate([r["vT"] for r in r1], axis=2).astype(np.float32)
    pT_full = np.concatenate([r["pT"] for r in r1], axis=2).astype(np.float32)
    pT_full[:, :, 2 * T - 1:] = 0

    pbu_c = f32(pos_bias_u).reshape(NH * DK).reshape(FC, P).T.copy()
    pbv_c = f32(pos_bias_v).reshape(NH * DK).reshape(FC, P).T.copy()
    quT_full = (qT_full + pbu_c[:, :, None]).astype(F8)
    qvT_full = qT_full + pbv_c[:, :, None]                    # f32
    pT_f8 = pT_full.astype(F8)

    wo_c = (_chunk_pf(f32(Wo)) * WSC).astype(F8)
    w1_c = _chunk_pf(f32(W1)).astype(BF)
    w2_c = np.ascontiguousarray(
        f32(W2).reshape(GC, P, H).transpose(1, 0, 2)).astype(BF)
    b1_c = f32(b1).reshape(GC, P).T.copy()
    if affine:
        vecs = np.stack([np.broadcast_to(f32(x), (P, H)) for x in
                         (bo, b2, ln1_g, ln1_b, ln2_g, ln2_b)], axis=1).copy()

    in2 = []
    for c in range(NCORE):
        b_ = c // 4
        q0 = 512 * (c % 4)
        w0 = 1536 - q0
        tsl = slice(T * b_, T * b_ + T)

        # zero-padded K=128 stationaries (fp8), partition dim first
        qvTz = np.zeros((P, NH, 4, P), F8)
        kTz = np.zeros((P, NH, 16, P), F8)
        for h in range(NH):
            hp, hcc = DK * (h % 2), h // 2
            qvTz[hp:hp + DK, h] = qvT_full[
                hp:hp + DK, hcc, 512 * c:512 * c + 512
            ].reshape(DK, 4, P).astype(F8)
            kTz[hp:hp + DK, h] = kT_full[
                hp:hp + DK, hcc, tsl].reshape(DK, 16, P).astype(F8)

        vv = vT_full[:, :, tsl]                                   # (128,6,2048)
        vmat = np.ascontiguousarray(
            vv.transpose(1, 0, 2).reshape(H, T))                  # (768,2048)=v.T
        arr = vmat.reshape(NH, DK, 16, P).transpose(0, 3, 2, 1)   # (12,128,16,64)
        # vb2[p, h, jp, t, d] = arr[h, p, 2*jp + t, d]; ones column at d=64
        vb_c = np.zeros((P, NH, 8, 2, 96), F8)
        vb_c[:, :, :, :, :DK] = arr.reshape(NH, P, 8, 2, DK).transpose(
            1, 0, 2, 3, 4).astype(F8)
        vb_c[:, :, :, :, DK] = 1.0
        entry = {
            "qvTz": qvTz,
            "pTw": np.ascontiguousarray(pT_f8[:, :, w0:w0 + WWIN]),
            "quT": np.ascontiguousarray(quT_full[:, :, 512 * c:512 * c + 512]),
            "kTz": kTz,
            "vb2": vb_c,
            "xq": np.ascontiguousarray(
                hf[T * b_ + q0: T * b_ + q0 + 512].reshape(4, P, H)
                .transpose(1, 0, 2)),
            "Wo": wo_c, "W1": w1_c, "W2": w2_c, "b1c": b1_c,
        }
        if affine:
            entry["vecs"] = vecs
        if use_mask:
            m = f32(mask[b_])
            mT = m.T[:, q0:q0 + 512]                              # (2048,512) j,q
            entry["maskb"] = (mT.reshape(16, P, 512).transpose(1, 0, 2)
                              * np.float32(-240.0)).astype(F8)
            entry["keepb"] = (1.0 - mT.reshape(16, P, 512)
                              .transpose(1, 0, 2)).astype(F8)
        in2.append(entry)

    PROFILE["in2"] = in2
    _res2 = run_bass_kernel_spmd(d2, in2, core_ids=list(range(NCORE)),
                                 trace=_trace)
    PROFILE["d2_ns"] = _res2.exec_time_ns
    PROFILE["d2_res"] = _res2
    r2 = _res2.results

    outp = np.zeros((B, T, H), np.float32)
    for c in range(NCORE):
        b_ = c // 4
        q0 = 512 * (c % 4)
        outp[b_, q0:q0 + 512] = r2[c]["out"].transpose(1, 0, 2).reshape(512, H)
    return outp
